# revision 9
# baseline (speedup 1.0000x reference)
"""Banded local-linear layer (nn_LocalLinearLayer) on 8 trn2 NeuronCores.

out[b, o, c] = sum_p W[o, p] * xpad[b, c, p] + bias[o],  band p in [o, o+25)
xpad = concat(x[:12], x, x[-12:]) along L (first/last 12 rows duplicated).

Strategy (v8, tensor-parallel over L, block-aligned x, warmed PE):
  - Each core owns 512 output rows (L/8); free dim = all B*C = 2048 cols.
    L-sharded weights keep replicated-weight HBM traffic tiny (~160KB/core).
  - xpad viewed as 128-row blocks: out tile t (128 rows) contracts over
    xpad rows [128t, 128t+152) = block t (K=128 matmul) + first 24 rows of
    block t+1 (K=24 matmul) accumulating into the same PSUM chunk.
    Blocks are partition-aligned -> x is DMAd with ZERO duplication and
    output tiles sit on all 128 partitions (all 16 SDMA engines on stores).
  - PE HAM warm-up: ~5us of dummy matmuls on a zeroed scratch tile while
    the first x DMA is in flight, so real matmuls run at 2.4GHz not 1.2.
  - fp16 operands + fp16 output (fp32 PSUM accum, fp32 bias).
  - PSUM->SBUF + bias split between ScalarE activation / VectorE
    tensor_scalar; x + weights stream on the Sync HWDGE ring, bias/halo +
    per-tile output stores on the Scalar HWDGE ring so stores overlap the
    input stream.
"""

import sys

for _p in ("/opt/trn_rl_repo",):
    if _p not in sys.path:
        sys.path.insert(0, _p)

import numpy as np

import concourse.bass as bass
import concourse.tile as tile
from concourse import bacc, mybir
from concourse.bass_utils import run_bass_kernel_spmd

L = 4096
WIN = 25
PAD = (WIN - 1) // 2  # 12
PADDED = L + 2 * PAD  # 4120
B = 32
C = 64
NCORES = 8
P = 128
RPC = L // NCORES  # 512 output rows per core
NT = RPC // P  # 4 tiles of 128 rows per core
HALO = WIN - 1  # 24
NF = B * C  # 2048 free columns
NCH = 4
CHUNK = NF // NCH  # 512 (one PSUM bank of fp32)
NBLK = (PADDED + P - 1) // P  # 33 blocks of xpad rows
NWARM = 13  # dummy matmuls to flip the PE HAM clock gate to 2.4GHz

F32 = mybir.dt.float32
F16 = mybir.dt.float16


def _host_weights(W: np.ndarray, b: np.ndarray):
    """w1[c][k, t, m] = Wm[base+m, base+k]      (k in [0,128))
    w2[c][k, t, m] = Wm[base+m, base+128+k]     (k in [0,24))
    bias[c][m, t]  = b[base+m],   base = 512c + 128t."""
    o = np.arange(L)[:, None]
    p = np.arange(PADDED)[None, :]
    Wm = np.where((p >= o) & (p < o + WIN), W, 0.0).astype(np.float32)
    w1 = np.zeros((NCORES, P, NT, P), np.float16)
    w2 = np.zeros((NCORES, HALO, NT, P), np.float16)
    bias = np.zeros((NCORES, P, NT), np.float32)
    for c in range(NCORES):
        for t in range(NT):
            base = RPC * c + P * t
            w1[c, :, t, :] = Wm[base : base + P, base : base + P].T
            w2[c, :, t, :] = Wm[base : base + P, base + P : base + P + HALO].T
            bias[c, :, t] = b[base : base + P]
    return w1, w2, bias


def _host_x(x: np.ndarray):
    """x [B, L, C] f32 -> xh [P, NBLK, B*C] f16, xh[p, blk, f] = xpad[b, 128*blk+p, c]."""
    xp = np.concatenate([x[:, :PAD], x, x[:, -PAD:]], axis=1).astype(np.float16)
    xpb = np.zeros((B, NBLK * P, C), np.float16)
    xpb[:, :PADDED] = xp
    xh = xpb.reshape(B, NBLK, P, C).transpose(2, 1, 0, 3).reshape(P, NBLK, NF)
    return xh


def _build_nc():
    nc = bacc.Bacc("TRN2", target_bir_lowering=False, debug=False, num_devices=NCORES)
    xm_d = nc.dram_tensor("xm", [P, NT, NF], F16, kind="ExternalInput").ap()
    xe_d = nc.dram_tensor("xe", [HALO, NF], F16, kind="ExternalInput").ap()
    w1_d = nc.dram_tensor("w1", [P, NT, P], F16, kind="ExternalInput").ap()
    w2_d = nc.dram_tensor("w2", [HALO, NT, P], F16, kind="ExternalInput").ap()
    bias_d = nc.dram_tensor("bias", [P, NT], F32, kind="ExternalInput").ap()
    out_d = nc.dram_tensor("out", [P, NT, NF], F16, kind="ExternalOutput").ap()

    with tile.TileContext(nc) as tc:
        with (
            tc.tile_pool(name="main", bufs=1) as pool,
            tc.tile_pool(name="ps", bufs=8, space=bass.MemorySpace.PSUM) as pspool,
        ):
            w1_s = pool.tile([P, NT, P], F16)
            w2_s = pool.tile([HALO, NT, P], F16)
            bias_s = pool.tile([P, NT], F32)
            scr = pool.tile([P, CHUNK], F16)
            xs = [pool.tile([P, NF], F16, name=f"x{t}") for t in range(NT)]
            xs.append(pool.tile([HALO, NF], F16, name="xe"))
            obs = [pool.tile([P, NF], F16, name=f"o{t}") for t in range(NT)]

            # x blocks + big weights on the Sync ring (x0/x1 first so the
            # first real matmul can start ASAP).
            nc.sync.dma_start(xs[0][:], xm_d[:, 0, :])
            nc.sync.dma_start(xs[1][:], xm_d[:, 1, :])
            nc.sync.dma_start(w1_s[:], w1_d)
            nc.sync.dma_start(xs[2][:], xm_d[:, 2, :])
            nc.sync.dma_start(xs[3][:], xm_d[:, 3, :])
            # small tensors + stores on the Scalar ring
            nc.scalar.dma_start(w2_s[:], w2_d)
            nc.scalar.dma_start(bias_s[:], bias_d)
            nc.scalar.dma_start(xs[NT][:], xe_d)

            # PE HAM warm-up: ~13 cold-clock dummy matmuls (~5.5us) while
            # the x stream is still in flight.
            nc.vector.memset(scr[:], 0.0)
            for _ in range(NWARM):
                wps = pspool.tile([P, CHUNK], F32, name="ps")
                nc.tensor.matmul(wps[:], scr[:, :P], scr[:], start=True, stop=True)

            for t in range(NT):
                for j in range(NCH):
                    sl = slice(j * CHUNK, (j + 1) * CHUNK)
                    ps = pspool.tile([P, CHUNK], F32)
                    nc.tensor.matmul(
                        ps[:], w1_s[:, t], xs[t][:, sl], start=True, stop=False
                    )
                    nc.tensor.matmul(
                        ps[:], w2_s[:, t], xs[t + 1][0:HALO, sl], start=False, stop=True
                    )
                    # ACT also issues the out DMAs -> ACT 7 : DVE 9 copies
                    if (t * NCH + j) % 16 in (0, 3, 5, 8, 10, 13, 15):
                        nc.scalar.activation(
                            obs[t][:, sl],
                            ps[:],
                            mybir.ActivationFunctionType.Identity,
                            bias=bias_s[:, t : t + 1],
                        )
                    else:
                        nc.vector.tensor_scalar_add(
                            obs[t][:, sl], ps[:], bias_s[:, t : t + 1]
                        )
                nc.scalar.dma_start(out_d[:, t, :], obs[t][:])

    nc.compile()
    return nc


_NC = None


def _get_nc():
    global _NC
    if _NC is None:
        _NC = _build_nc()
    return _NC


def _make_in_maps(x, W, b):
    w1, w2, bias = _host_weights(
        np.asarray(W, dtype=np.float32), np.asarray(b, dtype=np.float32)
    )
    xh = _host_x(np.asarray(x, dtype=np.float32))
    maps = []
    for c in range(NCORES):
        maps.append(
            {
                "xm": np.ascontiguousarray(xh[:, NT * c : NT * c + NT, :]),
                "xe": np.ascontiguousarray(xh[:HALO, NT * c + NT, :]),
                "w1": w1[c],
                "w2": w2[c],
                "bias": bias[c],
            }
        )
    return maps


def _gather(results):
    out = np.empty((B, L, C), np.float32)
    for c in range(NCORES):
        oh = results[c]["out"].astype(np.float32)  # [P, NT, NF]
        o4 = oh.reshape(P, NT, B, C).transpose(2, 1, 0, 3).reshape(B, RPC, C)
        out[:, RPC * c : RPC * (c + 1)] = o4
    return out


def kernel(x: np.ndarray, W: np.ndarray, b: np.ndarray) -> np.ndarray:
    nc = _get_nc()
    res = run_bass_kernel_spmd(nc, _make_in_maps(x, W, b), list(range(NCORES)))
    return _gather(res.results)


if __name__ == "__main__":
    rng = np.random.default_rng(0)
    x = rng.standard_normal((B, L, C), dtype=np.float32)
    W = rng.standard_normal((L, PADDED), dtype=np.float32) * 0.02
    b = rng.standard_normal((L,), dtype=np.float32) * 0.02
    print(kernel(x, W, b).shape)
